# revision 2
# baseline (speedup 1.0000x reference)
# Neural CDE kernel for 8 Trainium2 NeuronCores.
# Data-parallel over batch: 4096 samples -> 512 per core; the 255-step RK4
# scan runs locally per core; small MLP weights replicated.
#
# Per-core layout: feature-major activations zT [H=128 partitions, B free].
# Per RK4 eval: mm1/mm2 (128x128, f32r), synthetic softplus
# (softplus has no ACT table on this toolchain:
#   softplus(v) = relu(v) + poly4(exp(-|v|)) with poly4 ~ log1p on (0,1]),
# mm3 against W3 reordered c-outer, tanh, then the C=8 einsum as a fused
# multiply+prefix-sum custom DVE op with strided difference extraction.
import sys
sys.path.insert(0, '/opt/trn_rl_repo')

import numpy as np

B_FULL, N_INT, C, H = 4096, 255, 8, 128
N_CORES = 8
B_CORE = B_FULL // N_CORES          # 512
SUB = 2                             # subtiles over batch columns
BS = B_CORE // SUB                  # 256
N_STEPS_DEFAULT = 255

# log1p(t) ~ c1 t + c2 t^2 + c3 t^3 + c4 t^4 on [0,1], max abs err 7.2e-5
PC1, PC2, PC3, PC4 = 0.99745026, -0.47131188, 0.22570847, -0.05877124

_REGISTERED = {}


def _register_ops():
    if _REGISTERED:
        return _REGISTERED
    import concourse.dve_ops as dve_ops
    from concourse.dve_spec import (Spec, Src0, Src1, C0, C1, C2, C3, Zero,
                                    AluOp, scan, maxx, lower, _has_src1,
                                    _spill_c3_to_src1)
    from concourse.dve_uop import DveOpSpec

    def reg(name, spec, subdim=False):
        if name in dve_ops._SUB_OPCODE_FOR_NAME:
            return next(o for o in dve_ops.OPS if o.name == name)
        shas = {}
        for ver in ("v3", "v4"):
            s = DveOpSpec(name=name, opcode=0, uops=lower(spec, ver=ver),
                          rd1_en=_has_src1(spec))
            shas[ver] = s.sha(ver)
        op = dve_ops.DveOp(name, spec, subdim, uops_sha=shas)
        dve_ops.OPS.append(op)
        dve_ops.CUSTOM_DVE_SPECS[name] = spec
        dve_ops._SUB_OPCODE_FOR_NAME[name] = max(
            dve_ops._SUB_OPCODE_FOR_NAME.values()) + 1
        return op

    def _scanmul_ref(in0, in1, s0, s1, imm2):
        a = np.ascontiguousarray(in0).reshape(in0.shape[0], -1)
        b = np.ascontiguousarray(in1).reshape(in1.shape[0], -1)
        p = a.astype(np.float32) * b.astype(np.float32)
        return np.cumsum(p, axis=1).astype(np.float32)

    def _poly4_ref(in0, in1, s0, s1, imm2):
        t = in0.astype(np.float32)
        c1 = in1.reshape(in0.shape[0], 1) if hasattr(in1, 'reshape') else in1
        return (((s0 * t + s1) * t + imm2) * t + c1) * t

    def _sprelu_ref(in0, in1, s0, s1, imm2):
        return np.maximum(in0.astype(np.float32) + s0, 0) + in1

    _REGISTERED['SCANMUL'] = reg(
        "SCANMUL_ANT", Spec(body=scan(AluOp.ADD, Src0 * Src1),
                            reference=_scanmul_ref))
    _REGISTERED['POLY4'] = reg(
        "SPTAIL_ANT",
        Spec(body=_spill_c3_to_src1(
            (((Src0 * C0 + C1) * Src0 + C2) * Src0 + C3) * Src0),
            reference=_poly4_ref))
    _REGISTERED['SPRELU'] = reg(
        "SPRELU_ANT", Spec(body=maxx(Src0 + C0, Zero) + Src1,
                           reference=_sprelu_ref))
    return _REGISTERED


_NC_CACHE = {}


def build(n_steps=N_STEPS_DEFAULT):
    if n_steps in _NC_CACHE:
        return _NC_CACHE[n_steps]
    import concourse.mybir as mybir
    import concourse.tile as tile
    from concourse import bacc

    ops = _register_ops()
    SCANMUL, POLY4, SPRELU = ops['SCANMUL'], ops['POLY4'], ops['SPRELU']

    f32 = mybir.dt.float32
    f32r = mybir.dt.float32r
    AF = mybir.ActivationFunctionType
    OP = mybir.AluOpType

    nc = bacc.Bacc()
    coeffs_d = nc.dram_tensor("coeffs", [B_CORE, N_INT, 32], f32,
                              kind="ExternalInput")
    W1_d = nc.dram_tensor("W1", [H, H], f32, kind="ExternalInput")
    W2_d = nc.dram_tensor("W2", [H, H], f32, kind="ExternalInput")
    W3r_d = nc.dram_tensor("W3r", [H, H * C], f32, kind="ExternalInput")
    b1_d = nc.dram_tensor("b1c", [H, 1], f32, kind="ExternalInput")
    b2_d = nc.dram_tensor("b2c", [H, 1], f32, kind="ExternalInput")
    b3r_d = nc.dram_tensor("b3r", [1, C * H], f32, kind="ExternalInput")
    Wi_d = nc.dram_tensor("Wi", [C, H], f32, kind="ExternalInput")
    bi_d = nc.dram_tensor("bic", [H, 1], f32, kind="ExternalInput")
    Wo_d = nc.dram_tensor("Wo", [H, 1], f32, kind="ExternalInput")
    bo_d = nc.dram_tensor("bo", [1, 1], f32, kind="ExternalInput")
    ones_d = nc.dram_tensor("ones", [1, B_CORE], f32, kind="ExternalInput")
    out_d = nc.dram_tensor("out", [B_CORE, 1], f32, kind="ExternalOutput")

    with tile.TileContext(nc) as tc:
        with tc.tile_pool(name="const", bufs=1) as cp, \
             tc.tile_pool(name="state", bufs=3) as stp, \
             tc.tile_pool(name="work", bufs=2) as wp, \
             tc.tile_pool(name="tanh", bufs=2) as thp, \
             tc.tile_pool(name="scan", bufs=2) as scp, \
             tc.tile_pool(name="kpool", bufs=3) as kp, \
             tc.tile_pool(name="xdrep", bufs=3) as xp, \
             tc.tile_pool(name="xdram", bufs=4, space="DRAM") as dp, \
             tc.tile_pool(name="ps12", bufs=3, space="PSUM") as ps12, \
             tc.tile_pool(name="ps3", bufs=1, space="PSUM") as ps3:

            # ---- constants ----
            W1_t = cp.tile([H, H], f32r, tag="w1")
            W2_t = cp.tile([H, H], f32r, tag="w2")
            W3_t = cp.tile([H, H * C], f32r, tag="w3")
            b3r_t = cp.tile([1, C * H], f32r, tag="b3r")
            Wi_t = cp.tile([C, H], f32r, tag="wi")
            Wo_t = cp.tile([H, 1], f32r, tag="wo")
            b1_t = cp.tile([H, 1], f32, tag="b1")
            b2_t = cp.tile([H, 1], f32, tag="b2")
            bi_t = cp.tile([H, 1], f32, tag="bi")
            bo_t = cp.tile([1, 1], f32, tag="bo")
            ones_t = cp.tile([1, B_CORE], f32r, tag="ones")
            c1_t = cp.tile([H, 1], f32, tag="c1poly")
            for t_, d_ in [(W1_t, W1_d), (W2_t, W2_d), (W3_t, W3r_d),
                           (b3r_t, b3r_d), (Wi_t, Wi_d), (Wo_t, Wo_d)]:
                nc.gpsimd.dma_start(t_[:], d_[:])
            for t_, d_ in [(b1_t, b1_d), (b2_t, b2_d), (bi_t, bi_d),
                           (bo_t, bo_d)]:
                nc.sync.dma_start(t_[:], d_[:])
            nc.gpsimd.dma_start(ones_t[:], ones_d[:])
            nc.vector.memset(c1_t[:], PC1)

            coeffs_r = coeffs_d[:].rearrange("(bh p) t j -> p bh t j", p=H)

            # ---- z0 = a[:,0] @ W_init + b_init  (feature-major [H, B]) ----
            a0_t = cp.tile([C, B_CORE], f32r, tag="a0")
            nc.gpsimd.dma_start(
                a0_t[:], coeffs_d[:, 0, 0:C].rearrange("b c -> c b"))
            zps = ps3.tile([H, B_CORE], f32, tag="p3")
            nc.tensor.matmul(zps[:], Wi_t[:], a0_t[:], start=True, stop=True)
            zT = stp.tile([H, B_CORE], f32r, tag="z")
            nc.scalar.activation(zT[:], zps[:], AF.Identity, bias=bi_t[:])

            # ---- XdRep for step-0 k1: Xd = b-coef (frac 0) ----
            def stage(src_ap):
                """src [128,(4,8)] laid (p, bh, c) -> DRAM [1,4096] with
                col order b*8+c, b = bh*128 + p."""
                xdr = dp.tile([1, B_CORE * C], f32, tag="xdram")
                dst = xdr[0, :].rearrange("(bh p c) -> p bh c", p=H, bh=SUB * 2,
                                          c=C)
                nc.sync.dma_start(dst, src_ap)
                return xdr

            def bcast(xdr):
                rep = xp.tile([H, B_CORE * C], f32, tag="xdrep")
                nc.sync.dma_start(rep[:],
                                  xdr[0:1, :].to_broadcast((H, B_CORE * C)))
                return rep

            cf0 = wp.tile([H, 4, C], f32, tag="cf0")
            nc.sync.dma_start(cf0[:], coeffs_r[:, :, 0, C:2 * C])
            xd_prev = bcast(stage(cf0[:]))

            OPm, OPa, OPs = OP.mult, OP.add, OP.subtract

            def linear_f32r(name_tag, W_t, rhs_ap, bank_tag):
                ps = ps12.tile([H, BS], f32, tag=bank_tag)
                nc.tensor.matmul(ps[:], W_t[:], rhs_ap, start=True, stop=True)
                return ps

            def softplus(ps, b_t, s):
                a_t = wp.tile([H, BS], f32, tag=f"spa{s}")
                nc.scalar.activation(a_t[:], ps[:], AF.Abs, bias=b_t[:])
                e_t = wp.tile([H, BS], f32, tag=f"spe{s}")
                nc.scalar.activation(e_t[:], a_t[:], AF.Exp, scale=-1.0)
                g_t = wp.tile([H, BS], f32, tag=f"spg{s}")
                nc.vector._custom_dve(POLY4, out=g_t[:], in0=e_t[:],
                                      in1=c1_t[:], s0=PC4, s1=PC3, imm2=PC2)
                h_t = wp.tile([H, BS], f32r, tag=f"sph{s}")
                nc.vector._custom_dve(SPRELU, out=h_t[:], in0=ps[:],
                                      in1=g_t[:], s0=b_t[:, 0:1])
                return h_t

            def ode_f(z_sub_aps, xd_rep, step, ev):
                """One ode_f eval. z_sub_aps: per-subtile rhs APs [H, BS]
                (f32r). Returns per-subtile k tiles [H, BS] (f32)."""
                ks = []
                for s in range(SUB):
                    p1 = linear_f32r("mm1", W1_t, z_sub_aps[s], "p12")
                    h1 = softplus(p1, b1_t, s)
                    p2 = linear_f32r("mm2", W2_t, h1[:], "p12")
                    h2 = softplus(p2, b2_t, s)
                    # mm3: 8 chunks + rank-1 bias into one [H, 8*BS] psum
                    p3 = ps3.tile([H, C * BS], f32, tag="p3")
                    for c in range(C):
                        sl = p3[:, c * BS:(c + 1) * BS]
                        nc.tensor.matmul(sl, W3_t[:, c * H:(c + 1) * H],
                                         h2[:], start=True, stop=False)
                        nc.tensor.matmul(sl, b3r_t[0:1, c * H:(c + 1) * H],
                                         ones_t[:, s * BS:(s + 1) * BS],
                                         start=False, stop=True)
                    tb = thp.tile([H, C * BS], f32, tag=f"tanh{s}")
                    nc.scalar.activation(tb[:], p3[:], AF.Tanh)
                    # einsum over c: fused mul+prefix-sum, then strided diff
                    S_t = scp.tile([H, 8 + C * BS], f32, tag=f"scan{s}")
                    nc.vector.memset(S_t[:, 0:8], 0.0)
                    xd_sl = xd_rep[:, s * C * BS:(s + 1) * C * BS]
                    nc.vector._custom_dve(
                        SCANMUL,
                        out=S_t[:, 8:].rearrange("p (b c) -> p b c", c=C),
                        in0=tb[:].rearrange("p (c b) -> p b c", c=C),
                        in1=xd_sl.rearrange("p (b c) -> p b c", c=C),
                    )
                    k_t = kp.tile([H, BS], f32, tag=f"k{s}")
                    nc.gpsimd.tensor_tensor(
                        k_t[:], S_t[:, 15:8 + C * BS:C],
                        S_t[:, 7:C * BS:C], OPs)
                    ks.append(k_t)
                return ks

            def xd_make(cf, frac, tag):
                """Xd = b + (tc + td*frac)*frac from coeff slab [H,(4,24)]."""
                t1 = wp.tile([H, 4, C], f32, tag=f"xt1{tag}")
                nc.vector.scalar_tensor_tensor(
                    t1[:], cf[:, :, 2 * C:3 * C], float(frac),
                    cf[:, :, C:2 * C], OPm, OPa)
                t2 = wp.tile([H, 4, C], f32, tag=f"xt2{tag}")
                nc.vector.scalar_tensor_tensor(
                    t2[:], t1[:], float(frac), cf[:, :, 0:C], OPm, OPa)
                return stage(t2[:])

            for step in range(n_steps):
                # coeff slab for interval `step`: b, 2c, 3d  [H, (4, 24)]
                cf = wp.tile([H, 4, 3 * C], f32, tag="cf")
                nc.sync.dma_start(cf[:], coeffs_r[:, :, step, C:4 * C])
                xd13 = xd_make(cf, 1.0 / 3.0, "a")
                xd23 = xd_make(cf, 2.0 / 3.0, "b")
                xd1 = xd_make(cf, 1.0, "c")

                zsubs = [zT[:, s * BS:(s + 1) * BS] for s in range(SUB)]
                k1 = ode_f(zsubs, xd_prev, step, 1)

                # y2 = z + k1/3
                y2 = []
                for s in range(SUB):
                    y = wp.tile([H, BS], f32r, tag=f"y2{s}")
                    nc.vector.scalar_tensor_tensor(
                        y[:], k1[s][:], 1.0 / 3.0, zsubs[s], OPm, OPa)
                    y2.append(y[:])
                k2 = ode_f(y2, bcast(xd13), step, 2)

                # y3 = z + (k2 - k1/3)
                y3 = []
                for s in range(SUB):
                    t_ = wp.tile([H, BS], f32, tag=f"y3t{s}")
                    nc.vector.scalar_tensor_tensor(
                        t_[:], k1[s][:], -1.0 / 3.0, k2[s][:], OPm, OPa)
                    y = wp.tile([H, BS], f32r, tag=f"y3{s}")
                    nc.gpsimd.tensor_tensor(y[:], t_[:], zsubs[s], OPa)
                    y3.append(y[:])
                k3 = ode_f(y3, bcast(xd23), step, 3)

                # y4 = z + (k1 - k2 + k3)
                y4 = []
                for s in range(SUB):
                    t_ = wp.tile([H, BS], f32, tag=f"y4t{s}")
                    nc.vector.scalar_tensor_tensor(
                        t_[:], k2[s][:], -1.0, k1[s][:], OPm, OPa)
                    t2_ = wp.tile([H, BS], f32, tag=f"y4u{s}")
                    nc.gpsimd.tensor_tensor(t2_[:], t_[:], k3[s][:], OPa)
                    y = wp.tile([H, BS], f32r, tag=f"y4{s}")
                    nc.gpsimd.tensor_tensor(y[:], t2_[:], zsubs[s], OPa)
                    y4.append(y[:])
                xd1r = bcast(xd1)
                k4 = ode_f(y4, xd1r, step, 4)

                # z' = z + (k1 + 3k2 + 3k3 + k4)/8
                zn = stp.tile([H, B_CORE], f32r, tag="z")
                for s in range(SUB):
                    s1 = wp.tile([H, BS], f32, tag=f"zs1{s}")
                    nc.vector.scalar_tensor_tensor(
                        s1[:], k2[s][:], 3.0, k1[s][:], OPm, OPa)
                    s2 = wp.tile([H, BS], f32, tag=f"zs2{s}")
                    nc.vector.scalar_tensor_tensor(
                        s2[:], k3[s][:], 3.0, k4[s][:], OPm, OPa)
                    s3 = wp.tile([H, BS], f32, tag=f"zs3{s}")
                    nc.gpsimd.tensor_tensor(s3[:], s1[:], s2[:], OPa)
                    nc.vector.scalar_tensor_tensor(
                        zn[:, s * BS:(s + 1) * BS], s3[:], 1.0 / 8.0,
                        zsubs[s], OPm, OPa)
                zT = zn
                xd_prev = xd1r

            # ---- out = zT @ W_out + b_out ----
            ops_ = ps3.tile([1, B_CORE], f32, tag="p3")
            nc.tensor.matmul(ops_[:], Wo_t[:], zT[:], start=True, stop=True)
            ot = cp.tile([1, B_CORE], f32, tag="outs")
            nc.scalar.activation(ot[:], ops_[:], AF.Identity, bias=bo_t[:])
            nc.sync.dma_start(out_d[:].rearrange("b one -> one b"),
                              ot[:])

    nc.finalize()
    _NC_CACHE[n_steps] = nc
    return nc


def host_inputs(inputs, core):
    coeffs = np.ascontiguousarray(inputs["coeffs"][core * B_CORE:(core + 1) * B_CORE])
    W3 = inputs["W3"]
    W3r = np.ascontiguousarray(
        W3.reshape(H, H, C).transpose(0, 2, 1).reshape(H, H * C))
    b3r = np.ascontiguousarray(inputs["b3"].reshape(H, C).T.reshape(1, H * C))
    return dict(
        coeffs=coeffs.astype(np.float32),
        W1=inputs["W1"].astype(np.float32),
        W2=inputs["W2"].astype(np.float32),
        W3r=W3r.astype(np.float32),
        b1c=inputs["b1"].reshape(H, 1).astype(np.float32),
        b2c=inputs["b2"].reshape(H, 1).astype(np.float32),
        b3r=b3r.astype(np.float32),
        Wi=inputs["W_init"].astype(np.float32),
        bic=inputs["b_init"].reshape(H, 1).astype(np.float32),
        Wo=inputs["W_out"].reshape(H, 1).astype(np.float32),
        bo=inputs["b_out"].reshape(1, 1).astype(np.float32),
        ones=np.ones((1, B_CORE), np.float32),
    )


def kernel(**inputs):
    return _run(N_STEPS_DEFAULT, False, inputs)


def _run(n_steps, trace, inputs):
    from concourse.bass_utils import run_bass_kernel_spmd
    nc = build(n_steps)
    in_maps = [host_inputs(inputs, i) for i in range(N_CORES)]
    res = run_bass_kernel_spmd(nc, in_maps, core_ids=list(range(N_CORES)),
                               trace=trace)
    out = np.concatenate([res.results[i]["out"] for i in range(N_CORES)],
                         axis=0)
    _run.last_result = res
    return out



# revision 4
# speedup vs baseline: 1.2191x; 1.2191x over previous
# Neural CDE kernel for 8 Trainium2 NeuronCores — v2.
#
# Data-parallel over batch: 4096 -> 512 per core. Inside a core, the 512
# samples split into TWO INDEPENDENT STREAMS of 256 whose 255-step RK4
# scans are software-pipelined HALF AN EVAL apart: bracket k runs the back
# half (mm3 + tanh + scan + k-extract) of one stream's eval together with
# the front half (mm1 + softplus x2) of the other stream's, interleaved
# stage-by-stage in emission order (engine queues are FIFO).
#
# Per eval (feature-major [H=128 partitions, B free]):
#   p1 = W1^T y_partial + W1x^T kQ_prev  (the RK4 y-update's last term is
#   folded into PSUM accumulation with prescaled W1 variants; the rest of
#   the y combination runs on GPSIMD off the critical path);
#   softplus = Abs(+b bias) -> Exp (ACT LUTs) -> ONE fused DVE op
#   relu(p1+b) + t*(1 + t*(c2 + c3 t)), t = e^-|p1+b|; same for layer 2;
#   mm3 into the stream's [128, 2048] PSUM region (4 banks; p1/p2 are
#   column slices of it; per 2KB bank: chunk(start) + chunk(fresh) +
#   K=8 block-indicator bias matmul(stop)), tanh per bank (ACT, bf16),
#   the C=8 einsum as fused mul+prefix-scan DVE ops vs a broadcast xd row
#   (P/Q chunk halves), k halves via strided differences on GPSIMD.
# xd spline derivatives for all steps/fracs and z0 are precomputed on the
# host; xd rows are broadcast-DMA'd bf16 and prefetched a step ahead.
import sys
sys.path.insert(0, '/opt/trn_rl_repo')

import numpy as np

B_FULL, N_INT, C, H = 4096, 255, 8, 128
N_CORES = 8
B_CORE = B_FULL // N_CORES          # 512
NSTR = 2                            # independent pipelined streams
BS = B_CORE // NSTR                 # 256
CB = C * BS                         # 2048 (per-stream mm3/psum width)
HB = CB // 2                        # 1024 (P/Q half)
N_STEPS_DEFAULT = 255

# log1p(t) ~ t*(1 + t*(P2 + P3 t)) on (0,1], max abs err 1.9e-3
P2, P3 = -0.44687101, 0.14191479

_REGISTERED = {}


def _register_ops():
    if _REGISTERED:
        return _REGISTERED
    import concourse.dve_ops as dve_ops
    from concourse.dve_spec import (Spec, Src0, Src1, C0, C1, C2, Zero, One,
                                    AluOp, scan, maxx, sq, lower, _has_src1)
    from concourse.dve_uop import DveOpSpec

    def reg(name, spec, subdim=False):
        if name in dve_ops._SUB_OPCODE_FOR_NAME:
            return next(o for o in dve_ops.OPS if o.name == name)
        shas = {}
        for ver in ("v3", "v4"):
            s = DveOpSpec(name=name, opcode=0, uops=lower(spec, ver=ver),
                          rd1_en=_has_src1(spec))
            shas[ver] = s.sha(ver)
        op = dve_ops.DveOp(name, spec, subdim, uops_sha=shas)
        dve_ops.OPS.append(op)
        dve_ops.CUSTOM_DVE_SPECS[name] = spec
        dve_ops._SUB_OPCODE_FOR_NAME[name] = max(
            dve_ops._SUB_OPCODE_FOR_NAME.values()) + 1
        return op

    def _scanmul_ref(in0, in1, s0, s1, imm2):
        a = np.ascontiguousarray(in0).reshape(in0.shape[0], -1)
        b = np.ascontiguousarray(in1).reshape(in1.shape[0], -1)
        p = a.astype(np.float32) * b.astype(np.float32)
        return np.cumsum(p, axis=1).astype(np.float32)

    def _spfuse_ref(in0, in1, s0, s1, imm2):
        # relu(in1 + s0) + t + t^2 (imm2 + s1 t),  t = in0
        t = np.ascontiguousarray(in0).reshape(in0.shape[0], -1).astype(np.float32)
        x = np.ascontiguousarray(in1).reshape(in1.shape[0], -1).astype(np.float32)
        b = np.asarray(s0, np.float32)
        if b.ndim > 1:
            b = b.reshape(b.shape[0], 1)
        return (np.maximum(x + b, 0.0) + t + t * t * (imm2 + s1 * t)
                ).astype(np.float32)

    _REGISTERED['SCANMUL'] = reg(
        "SCANMUL_ANT", Spec(body=scan(AluOp.ADD, Src0 * Src1),
                            reference=_scanmul_ref))
    _REGISTERED['SPFUSE'] = reg(
        "SPFUSE2_ANT",
        Spec(body=(maxx(Src1 + C0, Zero) + Src0)
             + sq(Src0) * (Src0 * C1 + C2),
             reference=_spfuse_ref))

    def _absb_ref(in0, in1, s0, s1, imm2):
        x = np.ascontiguousarray(in0).reshape(in0.shape[0], -1).astype(np.float32)
        b = np.asarray(s0, np.float32)
        if b.ndim > 1:
            b = b.reshape(b.shape[0], 1)
        return np.abs(x + b).astype(np.float32)

    _REGISTERED['ABSB'] = reg(
        "ABSB_ANT",
        Spec(body=maxx(Src0 + C0, Zero - (Src0 + C0)),
             reference=_absb_ref))
    return _REGISTERED


_NC_CACHE = {}

# bracket interleave: f = front stage of one stream, b = back stage of the
# other (half-eval offset).  f: 0 mm1, 1 abs1, 2 exp1, 3 tail1, 4 mm2,
# 5 abs2, 6 exp2, 7 tail2.  b: 0 mm3bank0, 1 tanh0, 2 mm3bank1, 3 tanh1,
# 4 mm3bank2, 5 tanh2, 6 mm3bank3, 7 tanh3, 8 memsets, 9 scanP,
# 10 diffP+pool, 11 scanQ, 12 diffQ+pool.
PATTERN = ["f0", "b0", "f1", "b1", "b2", "f2", "b3", "f3", "b4", "b5",
           "f4", "b6", "b7", "f5", "f6", "b8", "b9", "b10", "b11", "f7",
           "b12"]
USE_ABSB = False     # layer-1 abs on DVE (custom op) instead of ACT
MEMSET_ONCE = False  # only zero scan pads on first use of each buffer


def build(n_steps=N_STEPS_DEFAULT):
    if n_steps in _NC_CACHE:
        return _NC_CACHE[n_steps]
    import concourse.mybir as mybir
    import concourse.tile as tile
    from concourse import bacc

    ops = _register_ops()
    SCANMUL, SPFUSE, ABSB = ops['SCANMUL'], ops['SPFUSE'], ops['ABSB']

    f32 = mybir.dt.float32
    f32r = mybir.dt.float32r
    bf16 = mybir.dt.bfloat16
    AF = mybir.ActivationFunctionType
    OP = mybir.AluOpType
    OPm, OPa, OPs = OP.mult, OP.add, OP.subtract

    n_rows = 1 + 3 * n_steps

    nc = bacc.Bacc()
    W1_d = nc.dram_tensor("W1", [H, H], f32, kind="ExternalInput")
    W13_d = nc.dram_tensor("W13", [H, H], f32, kind="ExternalInput")
    W18_d = nc.dram_tensor("W18", [H, H], f32, kind="ExternalInput")
    W2_d = nc.dram_tensor("W2", [H, H], f32, kind="ExternalInput")
    W3r_d = nc.dram_tensor("W3r", [H, H * C], f32, kind="ExternalInput")
    b1c_d = nc.dram_tensor("b1c", [H, 1], f32, kind="ExternalInput")
    b2c_d = nc.dram_tensor("b2c", [H, 1], f32, kind="ExternalInput")
    b3t8_d = nc.dram_tensor("b3t8", [C, H], f32, kind="ExternalInput")
    blk_d = nc.dram_tensor("blk", [C, CB], f32, kind="ExternalInput")
    z0_d = nc.dram_tensor("z0T", [H, B_CORE], f32, kind="ExternalInput")
    xdtab_d = nc.dram_tensor("xdtab", [n_rows, B_CORE * C], bf16,
                             kind="ExternalInput")
    Wo_d = nc.dram_tensor("Wo", [H, 1], f32, kind="ExternalInput")
    bo_d = nc.dram_tensor("bo", [1, 1], f32, kind="ExternalInput")
    out_d = nc.dram_tensor("out", [B_CORE, 1], f32, kind="ExternalOutput")

    with tile.TileContext(nc) as tc:
        with tc.tile_pool(name="const", bufs=1) as cp, \
             tc.tile_pool(name="work", bufs=2) as wp, \
             tc.tile_pool(name="tbp", bufs=2) as tbp, \
             tc.tile_pool(name="scp", bufs=2) as scp, \
             tc.tile_pool(name="kp", bufs=2) as kp, \
             tc.tile_pool(name="xp", bufs=2) as xp, \
             tc.tile_pool(name="zp", bufs=2) as zp, \
             tc.tile_pool(name="pp", bufs=1, space="PSUM") as pp:

            # ---- constants ----
            W1_t = cp.tile([H, H], f32r, tag="w1")
            W13_t = cp.tile([H, H], f32r, tag="w13")
            W18_t = cp.tile([H, H], f32r, tag="w18")
            W2_t = cp.tile([H, H], f32r, tag="w2")
            W3_t = cp.tile([H, H * C], f32r, tag="w3")
            b1_t = cp.tile([H, 1], f32, tag="b1")
            b2_t = cp.tile([H, 1], f32, tag="b2")
            b3_t = cp.tile([C, H], f32r, tag="b3")
            blk_t = cp.tile([C, CB], f32r, tag="blk")
            z0_t = cp.tile([H, B_CORE], f32r, tag="z0")
            Wo_t = cp.tile([H, 1], f32r, tag="wo")
            bo_t = cp.tile([1, 1], f32, tag="bo")
            for t_, d_ in [(W1_t, W1_d), (W13_t, W13_d), (W18_t, W18_d),
                           (W2_t, W2_d), (W3_t, W3r_d), (blk_t, blk_d),
                           (z0_t, z0_d), (b3_t, b3t8_d), (Wo_t, Wo_d)]:
                nc.gpsimd.dma_start(t_[:], d_[:])
            for t_, d_ in [(b1_t, b1c_d), (b2_t, b2c_d), (bo_t, bo_d)]:
                nc.sync.dma_start(t_[:], d_[:])

            # kQ-term weight per eval index (kq comes from eval ev-1)
            W_kq = {1: W18_t, 2: W13_t, 3: W1_t, 4: W1_t}

            # ---- xd broadcast tiles ----
            xd_tiles = {}

            def bcast(row, tag, bufs):
                t = xp.tile([H, B_CORE * C], bf16, tag=tag, bufs=bufs,
                            name=f"xd{row}")
                nc.sync.dma_start(
                    t[:], xdtab_d[row:row + 1, :].to_broadcast(
                        (H, B_CORE * C)))
                return t

            def prefetch_step(s):
                if s >= n_steps or (s, 0) in xd_tiles:
                    return
                xd_tiles[(s, 0)] = bcast(1 + 3 * s + 0, "x13", 2)
                xd_tiles[(s, 1)] = bcast(1 + 3 * s + 1, "x23", 2)
                xd_tiles[(s, 2)] = bcast(1 + 3 * s + 2, "x1", 3)

            xd_tiles[(-1, 2)] = bcast(0, "x1", 3)   # step-0 k1: frac-0 row
            prefetch_step(0)

            STAG = ["X", "Y"]
            strm = [
                dict(z=z0_t[:, 0:BS], partial=None, kq=None, k={},
                     kPQ={}, ps=None, h2=None, xd=None),
                dict(z=z0_t[:, BS:2 * BS], partial=None, kq=None, k={},
                     kPQ={}, ps=None, h2=None, xd=None),
            ]

            def xd_for(step, ev):
                if ev == 1:
                    return xd_tiles[(step - 1, 2)]
                return xd_tiles[(step, ev - 2)]

            def front_gen(s, step, ev):
                st = strm[s]
                nm = STAG[s]
                st['xd'] = xd_for(step, ev)
                ps = pp.tile([H, CB], f32, tag=STAG[s], name=f"ps{nm}")
                st['ps'] = ps
                p1 = ps[:, 0:BS]
                p2 = ps[:, 4 * BS:5 * BS]
                if st['kq'] is None:
                    nc.tensor.matmul(p1, W1_t[:], st['z'],
                                     start=True, stop=True)
                else:
                    nc.tensor.matmul(p1, W1_t[:], st['partial'][:],
                                     start=True, stop=False)
                    nc.tensor.matmul(p1, W_kq[ev][:], st['kq'][:],
                                     start=False, stop=True)
                if s == 0 and ev == 1:
                    prefetch_step(step + 1)
                yield
                ab = wp.tile([H, BS], f32, tag=f"ab{s}", name=f"ab{nm}")
                if USE_ABSB:
                    nc.vector._custom_dve(ABSB, out=ab[:], in0=p1,
                                          s0=b1_t[:, 0:1])
                else:
                    nc.scalar.activation(ab[:], p1, AF.Abs, bias=b1_t[:])
                yield
                ex = wp.tile([H, BS], f32, tag=f"ex{s}", name=f"ex{nm}")
                nc.scalar.activation(ex[:], ab[:], AF.Exp, scale=-1.0)
                yield
                h1 = wp.tile([H, BS], f32r, tag=f"h1{s}", name=f"h1{nm}")
                nc.vector._custom_dve(SPFUSE, out=h1[:], in0=ex[:], in1=p1,
                                      s0=b1_t[:, 0:1], s1=P3, imm2=P2)
                yield
                nc.tensor.matmul(p2, W2_t[:], h1[:], start=True, stop=True)
                yield
                ab2 = wp.tile([H, BS], f32, tag=f"ab{s}", name=f"ab2{nm}")
                nc.scalar.activation(ab2[:], p2, AF.Abs, bias=b2_t[:])
                yield
                ex2 = wp.tile([H, BS], f32, tag=f"ex{s}", name=f"ex2{nm}")
                nc.scalar.activation(ex2[:], ab2[:], AF.Exp, scale=-1.0)
                yield
                h2 = wp.tile([H, BS], f32r, tag=f"h2{s}", name=f"h2{nm}")
                nc.vector._custom_dve(SPFUSE, out=h2[:], in0=ex2[:], in1=p2,
                                      s0=b2_t[:, 0:1], s1=P3, imm2=P2)
                st['h2'] = h2
                yield

            def back_gen(s, step, ev):
                st = strm[s]
                nm = STAG[s]
                ps = st['ps']
                h2 = st['h2']
                tb = tbp.tile([H, CB], bf16, tag=f"tb{s}", name=f"tb{nm}")
                for bank in range(4):
                    c0 = 2 * bank
                    for i, c in enumerate((c0, c0 + 1)):
                        nc.tensor.matmul(ps[:, c * BS:(c + 1) * BS],
                                         W3_t[:, c * H:(c + 1) * H],
                                         h2[:], start=(i == 0), stop=False)
                    sl = slice(bank * 512, (bank + 1) * 512)
                    nc.tensor.matmul(ps[:, sl], b3_t[:], blk_t[:, sl],
                                     start=False, stop=True)
                    yield
                    nc.scalar.activation(tb[:, sl], ps[:, sl], AF.Tanh)
                    if bank < 3:
                        yield
                yield
                SP_ = scp.tile([H, 4 + HB], f32, tag=f"sp{s}", name=f"SP{nm}")
                SQ_ = scp.tile([H, 4 + HB], f32, tag=f"sq{s}", name=f"SQ{nm}")
                st['nset'] = st.get('nset', 0) + 1
                if not MEMSET_ONCE or st['nset'] <= 2:
                    nc.vector.memset(SP_[:, 0:4], 0.0)
                    nc.vector.memset(SQ_[:, 0:4], 0.0)
                yield
                xd = st['xd'][:, s * CB:(s + 1) * CB]
                xd3 = xd.rearrange("p (b c) -> p b c", c=C)
                nc.vector._custom_dve(
                    SCANMUL,
                    out=SP_[:, 4:].rearrange("p (b c) -> p b c", c=4),
                    in0=tb[:, 0:HB].rearrange("p (c b) -> p b c", c=4),
                    in1=xd3[:, :, 0:4],
                )
                yield
                # diffP + off-chain pool work (partial y for the next eval)
                kP = kp.tile([H, BS], f32r, tag=f"kp{s}", name=f"kP{nm}")
                nc.gpsimd.tensor_tensor(
                    kP[:], SP_[:, 7:4 + HB:4], SP_[:, 3:HB:4], OPs)
                st['kPQ'][ev] = [kP, None]
                ks = st['k']
                part = zp.tile([H, BS], f32r, tag=f"pt{s}", name=f"pt{ev}{nm}")
                if ev == 1:
                    nc.vector.scalar_tensor_tensor(
                        part[:], kP[:], 1.0 / 3.0, st['z'], OPm, OPa)
                elif ev == 2:
                    zmk = wp.tile([H, BS], f32, tag=f"t{s}", name=f"zmk{nm}")
                    nc.vector.scalar_tensor_tensor(
                        zmk[:], ks[1][:], -1.0 / 3.0, st['z'], OPm, OPa)
                    nc.gpsimd.tensor_tensor(part[:], zmk[:], kP[:], OPa)
                elif ev == 3:
                    u1 = wp.tile([H, BS], f32, tag=f"t{s}", name=f"u1{nm}")
                    nc.gpsimd.tensor_tensor(u1[:], ks[1][:], ks[2][:], OPs)
                    u2 = wp.tile([H, BS], f32, tag=f"u{s}", name=f"u2{nm}")
                    nc.gpsimd.tensor_tensor(u2[:], u1[:], st['z'], OPa)
                    nc.gpsimd.tensor_tensor(part[:], u2[:], kP[:], OPa)
                else:
                    s1 = wp.tile([H, BS], f32, tag=f"t{s}", name=f"zs1{nm}")
                    nc.gpsimd.tensor_tensor(s1[:], ks[2][:], ks[3][:], OPa)
                    s2 = wp.tile([H, BS], f32, tag=f"u{s}", name=f"zs2{nm}")
                    nc.vector.scalar_tensor_tensor(
                        s2[:], s1[:], 3.0, ks[1][:], OPm, OPa)
                    s3 = wp.tile([H, BS], f32, tag=f"v{s}", name=f"zs3{nm}")
                    nc.vector.scalar_tensor_tensor(
                        s3[:], s2[:], 1.0 / 8.0, st['z'], OPm, OPa)
                    nc.vector.scalar_tensor_tensor(
                        part[:], kP[:], 1.0 / 8.0, s3[:], OPm, OPa)
                st['partial'] = part
                yield
                nc.vector._custom_dve(
                    SCANMUL,
                    out=SQ_[:, 4:].rearrange("p (b c) -> p b c", c=4),
                    in0=tb[:, HB:CB].rearrange("p (c b) -> p b c", c=4),
                    in1=xd3[:, :, 4:8],
                )
                yield
                kQ = kp.tile([H, BS], f32r, tag=f"kq{s}", name=f"kQ{nm}")
                nc.gpsimd.tensor_tensor(
                    kQ[:], SQ_[:, 7:4 + HB:4], SQ_[:, 3:HB:4], OPs)
                st['kq'] = kQ
                if ev < 4:
                    kd = kp.tile([H, BS], f32, tag=f"k{ev}{s}",
                                 name=f"k{ev}{nm}")
                    nc.gpsimd.tensor_tensor(kd[:], kP[:], kQ[:], OPa)
                    st['k'][ev] = kd
                else:
                    zn = zp.tile([H, BS], f32r, tag=f"z{s}", name=f"zn{nm}")
                    nc.vector.scalar_tensor_tensor(
                        zn[:], kQ[:], 1.0 / 8.0, st['partial'][:], OPm, OPa)
                    st['z'] = zn[:]
                yield

            def bracket(bjob, fjob):
                bg = back_gen(*bjob) if bjob else None
                fg = front_gen(*fjob) if fjob else None
                for slot in PATTERN:
                    g = fg if slot[0] == 'f' else bg
                    if g is not None:
                        next(g, None)
                for g in (fg, bg):
                    if g is not None:
                        for _ in g:
                            pass

            jobs = [(step, ev) for step in range(n_steps)
                    for ev in (1, 2, 3, 4)]
            bracket(None, (0, *jobs[0]))
            for j in range(len(jobs)):
                bracket((0, *jobs[j]), (1, *jobs[j]))
                if j + 1 < len(jobs):
                    bracket((1, *jobs[j]), (0, *jobs[j + 1]))
            bracket((1, *jobs[-1]), None)

            # ---- out = zT @ W_out + b_out ----
            ops_ = pp.tile([1, B_CORE], f32, tag="X", name="psout")
            nc.tensor.matmul(ops_[0:1, 0:BS], Wo_t[:], strm[0]['z'],
                             start=True, stop=True)
            nc.tensor.matmul(ops_[0:1, BS:2 * BS], Wo_t[:], strm[1]['z'],
                             start=True, stop=True)
            ot = cp.tile([1, B_CORE], f32, tag="outs")
            nc.scalar.activation(ot[:], ops_[0:1, :], AF.Identity,
                                 bias=bo_t[:])
            nc.sync.dma_start(out_d[:].rearrange("b one -> one b"), ot[:])

    nc.finalize()
    _NC_CACHE[n_steps] = nc
    return nc


def host_inputs(inputs, core, n_steps=None):
    import concourse.mybir as mybir
    bf16_np = mybir.dt.np(mybir.dt.bfloat16)
    if n_steps is None:
        n_steps = getattr(host_inputs, "_n_steps", N_STEPS_DEFAULT)
    f32 = np.float32
    coeffs = inputs["coeffs"][core * B_CORE:(core + 1) * B_CORE].astype(f32)
    a, bc, tc2, td3 = np.split(coeffs, 4, axis=2)   # each [B_CORE, N_INT, C]

    n_rows = 1 + 3 * n_steps
    xdtab = np.empty((n_rows, B_CORE * C), f32)
    xdtab[0] = bc[:, 0].reshape(-1)
    for s in range(n_steps):
        idx = min(s, N_INT - 1)
        for j, frac in enumerate((1.0 / 3.0, 2.0 / 3.0, 1.0)):
            xd = bc[:, idx] + (tc2[:, idx] + td3[:, idx] * f32(frac)) * f32(frac)
            xdtab[1 + 3 * s + j] = xd.reshape(-1)

    W1 = inputs["W1"].astype(f32)
    W3 = inputs["W3"].astype(f32)
    W3r = np.ascontiguousarray(
        W3.reshape(H, H, C).transpose(0, 2, 1).reshape(H, H * C))
    b3t8 = np.ascontiguousarray(inputs["b3"].astype(f32).reshape(H, C).T)
    blk = np.zeros((C, CB), f32)
    for j in range(C):
        blk[j, j * BS:(j + 1) * BS] = 1.0
    z0 = (coeffs[:, 0, 0:C] @ inputs["W_init"].astype(f32)
          + inputs["b_init"].astype(f32))
    return dict(
        W1=W1,
        W13=(W1 / 3.0).astype(f32),
        W18=(W1 / 8.0).astype(f32),
        W2=inputs["W2"].astype(f32),
        W3r=W3r,
        b1c=inputs["b1"].reshape(H, 1).astype(f32),
        b2c=inputs["b2"].reshape(H, 1).astype(f32),
        b3t8=b3t8,
        blk=blk,
        z0T=np.ascontiguousarray(z0.T),
        xdtab=xdtab.astype(bf16_np),
        Wo=inputs["W_out"].reshape(H, 1).astype(f32),
        bo=inputs["b_out"].reshape(1, 1).astype(f32),
    )


def kernel(**inputs):
    return _run(N_STEPS_DEFAULT, False, inputs)


def _run(n_steps, trace, inputs):
    from concourse.bass_utils import run_bass_kernel_spmd
    nc = build(n_steps)
    host_inputs._n_steps = n_steps
    in_maps = [host_inputs(inputs, i, n_steps) for i in range(N_CORES)]
    res = run_bass_kernel_spmd(nc, in_maps, core_ids=list(range(N_CORES)),
                               trace=trace)
    out = np.concatenate([res.results[i]["out"] for i in range(N_CORES)],
                         axis=0)
    _run.last_result = res
    return out
